# revision 49
# baseline (speedup 1.0000x reference)
"""Bilateral anti-alias filter on Trainium2, 8-core data parallel.

Full inputs: images [16,3,512,512] f32, spatial_kernel [5,5] f32.
Shards the batch over 8 NeuronCores (2 images each), runs a Bass/Tile
kernel per core, gathers the full output.

Math (per pixel, K=5, sigma_i=0.1), using pair symmetry over the 12
offsets t=(di,dj) with di>=0 lexicographically positive:

  d_t = p(x+t) - p(x)
  e_t = exp(-50 d_t^2)  computed as Derivative_Erf(sqrt(50) d)*sqrt(pi)/2
  u_t = e_t * d_t
  den(x) = s0 + sum_t [s+_t e_t + s-_t shift_t(e_t)]
  pa(x)  = sum_t [s+_t u_t - s-_t shift_t(u_t)]
  out = p + pa / den

shift_t realized on the TensorEngine via banded lhsT matmuls into PSUM
(spatial weights folded into the lhsT values).  Each core runs 9 band
units: 4 full 124-row bands per image plus one shared tail unit that
computes both images' last 16 rows at once.  PSUM drains through ScalarE
(Reciprocal table + bf16 copy), finals are bf16 on DVE, and the output
is stored bf16 and upcast on the host.
"""
import sys

sys.path.insert(0, "/opt/trn_rl_repo")

import math
import numpy as np
import ml_dtypes
from contextlib import ExitStack

import concourse.bass as bass
import concourse.tile as tile
from concourse import bacc, mybir
from concourse.bass_utils import run_bass_kernel_spmd

f32 = mybir.dt.float32
bf16 = mybir.dt.bfloat16
AF = mybir.ActivationFunctionType
Alu = mybir.AluOpType

N_CORES = 8
B_FULL, C, H, W = 16, 3, 512, 512
B_SH = B_FULL // N_CORES  # 2 images per core
KK = 5
PAD = KK // 2  # 2
SQ50 = float(np.sqrt(np.float32(50.0)))
C_DERF = 2.0 / math.sqrt(math.pi)  # Derivative_Erf(x) = C_DERF*exp(-x^2)
NOUT = 124  # output rows per band
NG = 128    # plane partitions (= NOUT + 4)
WB = W + 4  # 516: padded col buffer, tile col c <-> image col c-2
WIN = W + 2  # 514: per-pair plane window width

# 12 pairs (di, dj) with di >= 0, lexicographically positive.
# Ordered so each batch's FIRST half holds even-dj pairs (which read only
# the ibA copies): the first ACT half-batch never waits on the ibB DMAs.
PAIRS = [
    (0, 2), (1, -2), (0, 1), (1, -1),
    (1, 0), (1, 2), (1, 1), (2, -1),
    (2, -2), (2, 0), (2, 1), (2, 2),
]
BATCHES = [PAIRS[0:4], PAIRS[4:8], PAIRS[8:12]]
NB = 4  # pairs per batch


def _jbase(dj):
    """Image col of plane-window col 0 (window covers jbase..jbase+513)."""
    return -2 if dj > 0 else 0


def _act_recip(nc, out, in_, bias):
    """rec = 1/(in_ + bias) via the ACT Reciprocal table (bass's public
    activation() refuses Reciprocal; its accuracy is ~1e-5 rel on our
    [1, 10] domain, fine for this kernel's 2e-2 budget)."""
    bias_ap = nc.const_aps.scalar_like(float(bias), in_)
    ins = [
        nc.scalar.lower_ap(in_),
        nc.scalar.lower_ap(bias_ap),
        mybir.ImmediateValue(dtype=f32, value=1.0),  # scale
        mybir.ImmediateValue(dtype=f32, value=0.0),  # alpha
    ]
    return nc.scalar.add_instruction(
        mybir.InstActivation(
            name=nc.get_next_instruction_name(),
            func=AF.Reciprocal,
            ins=ins,
            outs=[nc.scalar.lower_ap(out)],
        )
    )


def _restrict_act_tables():
    """Steer the activation-table chooser so per-band table swaps stay at
    2 (derf set <-> recip set): keep every set (indices into act_info.json
    must be preserved) but strip Derivative_Erf/Reciprocal membership from
    all other sets so they can't be chosen for them."""
    import concourse.bacc as cbacc

    if getattr(cbacc.get_activation_tables, "_bilateral_patched", False):
        return
    orig = cbacc.get_activation_tables
    keep = {
        "erf_derivative",
        "natural_log_exp_and_others",
        "reciprocal_and_small",
    }
    strip = {AF.Exp, AF.Ln, AF.Derivative_Erf, AF.Reciprocal}

    def patched(arch):
        tabs = orig(arch)
        return {
            k: (set(v) if k in keep else set(v) - strip)
            for k, v in tabs.items()
        }

    patched._bilateral_patched = True
    cbacc.get_activation_tables = patched


def _shift_mats(spatial):
    """Banded lhsT matrices [NG, n_mats, NOUT] bf16 with spatial weights
    (divided by C_DERF) folded in. Returns (array, {(di,dj,kind): idx})."""
    def L(k, scale):
        a = np.zeros((NG, NOUT), np.float64)
        for m in range(NOUT):
            a[m + k, m] = scale
        return a

    mats, idx = [], {}
    idx["s0"] = 0
    mats.append(L(2, float(spatial[2, 2])))  # ones-stream: den += s0
    for (di, dj) in PAIRS:
        sp = float(spatial[2 + di, 2 + dj]) / C_DERF
        sm = float(spatial[2 - di, 2 - dj]) / C_DERF
        if dj == 0:
            idx[(di, dj, "den")] = len(mats)
            mats.append(L(2, sp) + L(2 - di, sm))
            idx[(di, dj, "num")] = len(mats)
            mats.append(L(2, sp) - L(2 - di, sm))
        else:
            idx[(di, dj, "A")] = len(mats)
            mats.append(L(2, sp))
            idx[(di, dj, "B")] = len(mats)
            mats.append(L(2 - di, sm))
            idx[(di, dj, "C")] = len(mats)
            mats.append(L(2 - di, -sm))
    arr = np.stack(mats, 1)  # [NG, n_mats, NOUT]
    return arr.astype(ml_dtypes.bfloat16), idx


N_MATS = 1 + 2 * 2 + 10 * 3  # 35


def _reflect_runs(v0, v1, h):
    """Split virtual row range [v0, v1] into runs of physical rows.
    Returns list of (p_offset, phys_start, count, step) with step +-1."""
    runs = []
    v = v0
    while v <= v1:
        if v < 0:
            e = min(-1, v1)
            runs.append((v - v0, -v, e - v + 1, -1))
            v = e + 1
        elif v >= h:
            e = v1
            runs.append((v - v0, 2 * h - 2 - v, e - v + 1, -1))
            v = e + 1
        else:
            e = min(h - 1, v1)
            runs.append((v - v0, v, e - v + 1, 1))
            v = e + 1
    return runs


def build_bilateral(nc, s0, mat_idx, h=H, w=W, b_sh=B_SH, c=C):
    """Emit the per-core program. s0 = spatial[2,2] (center weight)."""
    img_d = nc.dram_tensor("images", [b_sh, c, h, w], f32, kind="ExternalInput").ap()
    shifts_d = nc.dram_tensor(
        "shifts", [NG, N_MATS, NOUT], bf16, kind="ExternalInput"
    ).ap()
    out_d = nc.dram_tensor("out", [b_sh, c, h, w], bf16, kind="ExternalOutput").ap()

    # const APs for activation biases (0.0 for derf, s0 for Identity-add)
    for val in sorted({0.0, float(s0)}):
        key = (f32, val)
        if key not in nc.const_aps.aps:
            t = nc.alloc_sbuf_tensor(f"cbias-{val}", [128, 1], f32)
            nc.gpsimd.memset(t.ap(), val)
            nc.const_aps.aps[key] = t.ap()
    nc.all_engine_barrier()

    with tile.TileContext(nc) as tc, ExitStack() as ctx:
        consts = ctx.enter_context(tc.tile_pool(name="consts", bufs=1))
        imgs_f = ctx.enter_context(tc.tile_pool(name="imgs_f", bufs=2))
        imgs_b = ctx.enter_context(tc.tile_pool(name="imgs_b", bufs=2))
        dpool = ctx.enter_context(tc.tile_pool(name="dpool", bufs=3))
        gpool = ctx.enter_context(tc.tile_pool(name="gpool", bufs=3))
        upool = ctx.enter_context(tc.tile_pool(name="upool", bufs=3))
        finals = ctx.enter_context(tc.tile_pool(name="finals", bufs=1))
        psums = ctx.enter_context(tc.tile_pool(name="psums", bufs=1, space="PSUM"))

        shifts = consts.tile([NG, N_MATS, NOUT], bf16)

        # Units: 4 full 124-row bands per image (no overlap), then one
        # shared tail unit computing both images' last 16 rows at once
        # (img0 at plane partitions 0.., img1 at 32..). The banded lhsT
        # matrices are reused by column-slicing [:, idx, 0:mtot].
        units = []
        for bi in range(b_sh):
            for r0 in range(0, h - NOUT - 15, NOUT):
                units.append([(bi, r0, 0, NOUT)])
        tail_r0 = (h // NOUT) * NOUT  # 496
        units.append([(ii, tail_r0, 32 * ii, h - tail_r0) for ii in range(b_sh)])

        pending_drain = None
        pending_drain_vec = None
        for ui, segs in enumerate(units):
            mtot = max(pb + nr for (_, _, pb, nr) in segs)
            if True:
                # ---- load 3 row-shifted reflect-padded f32 image copies ----
                # First unit: nothing else runs yet, so alternate the load
                # DMAs across both HWDGE rings (SP + ACT) to double the
                # in-flight transfer budget and halve the cold-start ramp.
                nld = 0
                ifs = []
                for s in range(3):
                    t = imgs_f.tile([NG, c, WB], f32, tag=f"i{s}f")
                    for (bi, r0, pb, nr) in segs:
                        for (po, ps, cnt, step) in _reflect_runs(
                            r0 - 2 + s, r0 + s + nr + 1, h
                        ):
                            if step == 1:
                                # per-channel DMAs: each instruction rides
                                # one SDMA engine (~27 GiB/s), so splitting
                                # puts 3 engines on the tile; first unit
                                # splits rows too (cold start, empty lanes)
                                # row-split only at pipeline-restart units
                                # (cold DMA lanes); elsewhere fewer, larger
                                # DMAs keep the issue queue shorter
                                rsp = 2 if (r0 == 0 and cnt > 32) else 1
                                for ch in range(c):
                                    for ri in range(rsp):
                                        ra = po + (cnt * ri) // rsp
                                        rb = po + (cnt * (ri + 1)) // rsp
                                        sa = ps + (cnt * ri) // rsp
                                        ldq = (
                                            nc.scalar
                                            if ui == 0 and nld % 2 == 1
                                            else nc.sync
                                        )
                                        nld += 1
                                        ldq.dma_start(
                                            t[pb + ra : pb + rb, ch, 2 : 2 + w],
                                            img_d[bi, ch, sa : sa + rb - ra, :],
                                        )
                            else:
                                # reflected rows: load straight from DRAM
                                for k in range(cnt):
                                    nc.sync.dma_start(
                                        t[pb + po + k : pb + po + k + 1, :, 2 : 2 + w],
                                        img_d[
                                            bi, :, ps - k : ps - k + 1, :
                                        ].rearrange("c r n -> r c n"),
                                    )
                    # reflect pad cols (image cols -2,-1,512,513), tiny DVE
                    for (j, jsrc) in ((0, 4), (1, 3), (2 + w, w), (3 + w, w - 1)):
                        nc.vector.tensor_copy(
                            t[:, :, j : j + 1], t[:, :, jsrc : jsrc + 1]
                        )
                    ifs.append(t)
                    if ui == 0 and s == 1:
                        # the 1.1MB weights load, chunked (one dma_start
                        # rides one SDMA engine), emitted only now so the
                        # first unit's image loads win the issue queue;
                        # only the chunks batch 1 needs go here, the rest
                        # after the ibB copies
                        for i in range(0, 15, 5):
                            nc.sync.dma_start(
                                shifts[:, i : i + 5, :], shifts_d[:, i : i + 5, :]
                            )

                ibA, ibB = [], []
                for s in range(3):
                    a = imgs_b.tile([NG, c, WB], bf16, tag=f"i{s}bA")
                    nc.vector.tensor_copy(a[:], ifs[s][:])
                    ibA.append(a)
                for s in range(3):
                    b = imgs_b.tile([NG, c, WB], bf16, tag=f"i{s}bB")
                    for ch in range(c):
                        nc.sync.dma_start(
                            b[:, ch, 0 : WB - 1], ibA[s][:, ch, 1:WB]
                        )
                    ibB.append(b)
                if ui == 0:
                    for i in range(15, N_MATS, 5):
                        j2 = min(i + 5, N_MATS)
                        nc.sync.dma_start(
                            shifts[:, i:j2, :], shifts_d[:, i:j2, :]
                        )

                # all subs issued up-front so the in-order DVE queue never
                # blocks a later batch's subs behind an earlier batch's umult
                dtiles = []
                for bt, batch in enumerate(BATCHES):
                    d = dpool.tile([NG, NB * c, WIN], bf16, tag="d")
                    for sl, (di, dj) in enumerate(batch):
                        jb = _jbase(dj)
                        cen = ibA[0][:, :, 2 + jb : 2 + jb + WIN]
                        if dj % 2 == 0:
                            sh = ibA[di][:, :, 2 + jb + dj : 2 + jb + dj + WIN]
                        else:
                            sh = ibB[di][:, :, 1 + jb + dj : 1 + jb + dj + WIN]
                        dsl = d[:, sl * c : (sl + 1) * c, :]
                        nc.vector.tensor_tensor(dsl, sh, cen, Alu.subtract)
                    dtiles.append(d)

                # ---- PSUM accumulators ----
                pw = psums.tile([NOUT, c, 512], f32, tag="pw")
                pa = psums.tile([NOUT, c, 512], f32, tag="pa")

                # matmuls per psum bank (channel)
                n_pw_ch = 2 * 1 + 10 * 2
                n_pa_ch = 2 * 1 + 10 * 2
                pw_cnt = [0] * c
                pa_cnt = [0] * c
                deferred_pa = []

                for bt, batch in enumerate(BATCHES):
                    d = dtiles[bt]
                    g = gpool.tile([NG, NB * c, WIN], bf16, tag="g")
                    u = upool.tile([NG, NB * c, WIN], bf16, tag="u")
                    # half-batch ACT + umult for finer pipelining; the very
                    # first batch of the kernel runs per-pair ACTs so the
                    # first matmul is gated on a 1.6us ACT, not a 2.9us one
                    hh = NB * c // 2
                    if ui == 0 and bt == 0:
                        for q in range(0, hh, c):
                            nc.scalar.activation(
                                g[:, q : q + c, :], d[:, q : q + c, :],
                                AF.Derivative_Erf, bias=0.0, scale=SQ50,
                            )
                    else:
                        nc.scalar.activation(
                            g[:, 0:hh, :], d[:, 0:hh, :],
                            AF.Derivative_Erf, bias=0.0, scale=SQ50,
                        )
                    if bt == 0 and pending_drain is not None:
                        # previous unit's scalar-side drain (recip frees the
                        # pw banks, pasb frees pa) goes right behind this
                        # unit's first derf half: early PSUM release without
                        # its matmul-stop wait blocking the derfs
                        pending_drain()
                        pending_drain = None
                    nc.scalar.activation(
                        g[:, hh:, :], d[:, hh:, :],
                        AF.Derivative_Erf, bias=0.0, scale=SQ50,
                    )
                    nc.vector.tensor_tensor(
                        u[:, 0:hh, :], g[:, 0:hh, :], d[:, 0:hh, :], Alu.mult
                    )
                    nc.vector.tensor_tensor(
                        u[:, hh:, :], g[:, hh:, :], d[:, hh:, :], Alu.mult
                    )
                    if bt == 0 and pending_drain_vec is not None:
                        # previous unit's vector-side finals + stores go
                        # after this unit's u-mults on the DVE queue
                        pending_drain_vec()
                        pending_drain_vec = None

                    # ---- PE accumulation streams ----
                    # start/stop are per PSUM zero-region (= per channel bank)
                    def mm_pw(mat, rhs, ch):
                        k = pw_cnt[ch]
                        nc.tensor.matmul(
                            pw[0:mtot, ch, :], mat, rhs,
                            start=k == 0, stop=k == n_pw_ch - 1,
                        )
                        pw_cnt[ch] = k + 1

                    def mm_pa(mat, rhs, ch):
                        k = pa_cnt[ch]
                        nc.tensor.matmul(
                            pa[0:mtot, ch, :], mat, rhs,
                            start=k == 0, stop=k == n_pa_ch - 1,
                        )
                        pa_cnt[ch] = k + 1

                    # pw (g-dependent) streams first: PE can start right
                    # after the ACT, overlapping the u-mults on DVE
                    for sl, (di, dj) in enumerate(batch):
                        jb = _jbase(dj)
                        od = -jb            # direct window offset in plane
                        os_ = -jb - dj      # shifted window offset
                        if dj == 0:
                            for rhs_ch in range(c):
                                mm_pw(shifts[:, mat_idx[(di, dj, "den")], 0:mtot],
                                      g[:, sl * c + rhs_ch, od : od + 512],
                                      rhs_ch)
                        else:
                            for rhs_ch in range(c):
                                mm_pw(shifts[:, mat_idx[(di, dj, "A")], 0:mtot],
                                      g[:, sl * c + rhs_ch, od : od + 512],
                                      rhs_ch)
                            for rhs_ch in range(c):
                                mm_pw(shifts[:, mat_idx[(di, dj, "B")], 0:mtot],
                                      g[:, sl * c + rhs_ch, os_ : os_ + 512],
                                      rhs_ch)

                    def emit_pa(batch, u):
                        for sl, (di, dj) in enumerate(batch):
                            jb = _jbase(dj)
                            od = -jb
                            os_ = -jb - dj
                            if dj == 0:
                                for rhs_ch in range(c):
                                    mm_pa(
                                        shifts[:, mat_idx[(di, dj, "num")], 0:mtot],
                                        u[:, sl * c + rhs_ch, od : od + 512],
                                        rhs_ch)
                            else:
                                for rhs_ch in range(c):
                                    mm_pa(
                                        shifts[:, mat_idx[(di, dj, "A")], 0:mtot],
                                        u[:, sl * c + rhs_ch, od : od + 512],
                                        rhs_ch)
                                for rhs_ch in range(c):
                                    mm_pa(
                                        shifts[:, mat_idx[(di, dj, "C")], 0:mtot],
                                        u[:, sl * c + rhs_ch, os_ : os_ + 512],
                                        rhs_ch)

                    if ui == len(units) - 1:
                        # last unit: defer all pa streams behind all pw
                        # streams so the reciprocal (gated on pw) overlaps
                        # the remaining matmuls -> shorter end-of-kernel tail
                        deferred_pa.append((batch, u))
                        if bt == len(BATCHES) - 1:
                            for b_, u_ in deferred_pa:
                                emit_pa(b_, u_)
                    else:
                        emit_pa(batch, u)

                # ---- drain + finals, deferred into the next unit's
                # emission (or flushed at the end). Scalar phase first
                # (frees PSUM banks), vector phase later.
                def make_drain(pw, pa, cen, segs, mtot):
                    state = {}

                    def drain_scalar():
                        # rec = 1/(pw+s0) and pasb = bf16(pa) on ScalarE
                        rec = finals.tile([NOUT, c, 512], bf16, tag="rec")
                        _act_recip(nc, rec[0:mtot], pw[0:mtot], float(s0))
                        pasb = finals.tile([NOUT, c, 512], bf16, tag="pasb")
                        nc.scalar.copy(pasb[0:mtot], pa[0:mtot])
                        state["rec"], state["pasb"] = rec, pasb

                    def drain_vec():
                        rec, pasb = state["rec"], state["pasb"]
                        # out = p + pa * rec
                        res = finals.tile([NOUT, c, 512], bf16, tag="res")
                        nc.vector.tensor_tensor(
                            res[0:mtot], pasb[0:mtot], rec[0:mtot], Alu.mult
                        )
                        outp = finals.tile([NOUT, c, 512], bf16, tag="outp", bufs=2)
                        nc.vector.tensor_tensor(
                            outp[0:mtot], cen[0:mtot], res[0:mtot], Alu.add
                        )
                        for (bi, r0, pb, nr) in segs:
                            for ch in range(c):
                                nc.sync.dma_start(
                                    out_d[bi, ch, r0 : r0 + nr, :],
                                    outp[pb : pb + nr, ch, :],
                                )
                    return drain_scalar, drain_vec

                pending_drain, pending_drain_vec = make_drain(
                    pw, pa, ibA[2][:, :, 2 : 2 + w], segs, mtot
                )
        if pending_drain is not None:
            pending_drain()
        if pending_drain_vec is not None:
            pending_drain_vec()
    return nc


def make_program(spatial_kernel):
    spatial_kernel = np.asarray(spatial_kernel, dtype=np.float32)
    mats, mat_idx = _shift_mats(spatial_kernel)
    s0 = float(spatial_kernel[2, 2])
    _restrict_act_tables()
    nc = bacc.Bacc("TRN2", target_bir_lowering=False, debug=False)
    build_bilateral(nc, s0, mat_idx)
    nc.compile()
    return nc, mats


def kernel(images, spatial_kernel):
    images = np.asarray(images, dtype=np.float32)
    spatial_kernel = np.asarray(spatial_kernel, dtype=np.float32)
    nc, mats = make_program(spatial_kernel)
    in_maps = [
        {"images": images[i * B_SH : (i + 1) * B_SH], "shifts": mats}
        for i in range(N_CORES)
    ]
    res = run_bass_kernel_spmd(nc, in_maps, core_ids=list(range(N_CORES)))
    return np.concatenate(
        [res.results[i]["out"].astype(np.float32) for i in range(N_CORES)], axis=0
    )


# revision 50
# speedup vs baseline: 1.0068x; 1.0068x over previous
"""Bilateral anti-alias filter on Trainium2, 8-core data parallel.

Full inputs: images [16,3,512,512] f32, spatial_kernel [5,5] f32.
Shards the batch over 8 NeuronCores (2 images each), runs a Bass/Tile
kernel per core, gathers the full output.

Math (per pixel, K=5, sigma_i=0.1), using pair symmetry over the 12
offsets t=(di,dj) with di>=0 lexicographically positive:

  d_t = p(x+t) - p(x)
  e_t = exp(-50 d_t^2)  computed as Derivative_Erf(sqrt(50) d)*sqrt(pi)/2
  u_t = e_t * d_t
  den(x) = s0 + sum_t [s+_t e_t + s-_t shift_t(e_t)]
  pa(x)  = sum_t [s+_t u_t - s-_t shift_t(u_t)]
  out = p + pa / den

shift_t realized on the TensorEngine via banded lhsT matmuls into PSUM
(spatial weights folded into the lhsT values).  Each core runs 9 band
units: 4 full 124-row bands per image plus one shared tail unit that
computes both images' last 16 rows at once.  PSUM drains through ScalarE
(Reciprocal table + bf16 copy), finals are bf16 on DVE, and the output
is stored bf16 and upcast on the host.
"""
import sys

sys.path.insert(0, "/opt/trn_rl_repo")

import math
import numpy as np
import ml_dtypes
from contextlib import ExitStack

import concourse.bass as bass
import concourse.tile as tile
from concourse import bacc, mybir
from concourse.bass_utils import run_bass_kernel_spmd

f32 = mybir.dt.float32
bf16 = mybir.dt.bfloat16
AF = mybir.ActivationFunctionType
Alu = mybir.AluOpType

N_CORES = 8
B_FULL, C, H, W = 16, 3, 512, 512
B_SH = B_FULL // N_CORES  # 2 images per core
KK = 5
PAD = KK // 2  # 2
SQ50 = float(np.sqrt(np.float32(50.0)))
C_DERF = 2.0 / math.sqrt(math.pi)  # Derivative_Erf(x) = C_DERF*exp(-x^2)
NOUT = 124  # output rows per band
NG = 128    # plane partitions (= NOUT + 4)
WB = W + 4  # 516: padded col buffer, tile col c <-> image col c-2
WIN = W + 2  # 514: per-pair plane window width

# 12 pairs (di, dj) with di >= 0, lexicographically positive.
# Ordered so each batch's FIRST half holds even-dj pairs (which read only
# the ibA copies): the first ACT half-batch never waits on the ibB DMAs.
PAIRS = [
    (0, 2), (1, -2), (0, 1), (1, -1),
    (1, 0), (1, 2), (1, 1), (2, -1),
    (2, -2), (2, 0), (2, 1), (2, 2),
]
BATCHES = [PAIRS[0:4], PAIRS[4:8], PAIRS[8:12]]
NB = 4  # pairs per batch


def _jbase(dj):
    """Image col of plane-window col 0 (window covers jbase..jbase+513)."""
    return -2 if dj > 0 else 0


def _act_recip(nc, out, in_, bias):
    """rec = 1/(in_ + bias) via the ACT Reciprocal table (bass's public
    activation() refuses Reciprocal; its accuracy is ~1e-5 rel on our
    [1, 10] domain, fine for this kernel's 2e-2 budget)."""
    bias_ap = nc.const_aps.scalar_like(float(bias), in_)
    ins = [
        nc.scalar.lower_ap(in_),
        nc.scalar.lower_ap(bias_ap),
        mybir.ImmediateValue(dtype=f32, value=1.0),  # scale
        mybir.ImmediateValue(dtype=f32, value=0.0),  # alpha
    ]
    return nc.scalar.add_instruction(
        mybir.InstActivation(
            name=nc.get_next_instruction_name(),
            func=AF.Reciprocal,
            ins=ins,
            outs=[nc.scalar.lower_ap(out)],
        )
    )


def _restrict_act_tables():
    """Steer the activation-table chooser so per-band table swaps stay at
    2 (derf set <-> recip set): keep every set (indices into act_info.json
    must be preserved) but strip Derivative_Erf/Reciprocal membership from
    all other sets so they can't be chosen for them."""
    import concourse.bacc as cbacc

    if getattr(cbacc.get_activation_tables, "_bilateral_patched", False):
        return
    orig = cbacc.get_activation_tables
    keep = {
        "erf_derivative",
        "natural_log_exp_and_others",
        "reciprocal_and_small",
    }
    strip = {AF.Exp, AF.Ln, AF.Derivative_Erf, AF.Reciprocal}

    def patched(arch):
        tabs = orig(arch)
        return {
            k: (set(v) if k in keep else set(v) - strip)
            for k, v in tabs.items()
        }

    patched._bilateral_patched = True
    cbacc.get_activation_tables = patched


def _shift_mats(spatial):
    """Banded lhsT matrices [NG, n_mats, NOUT] bf16 with spatial weights
    (divided by C_DERF) folded in. Returns (array, {(di,dj,kind): idx})."""
    def L(k, scale):
        a = np.zeros((NG, NOUT), np.float64)
        for m in range(NOUT):
            a[m + k, m] = scale
        return a

    mats, idx = [], {}
    idx["s0"] = 0
    mats.append(L(2, float(spatial[2, 2])))  # ones-stream: den += s0
    for (di, dj) in PAIRS:
        sp = float(spatial[2 + di, 2 + dj]) / C_DERF
        sm = float(spatial[2 - di, 2 - dj]) / C_DERF
        if dj == 0:
            idx[(di, dj, "den")] = len(mats)
            mats.append(L(2, sp) + L(2 - di, sm))
            idx[(di, dj, "num")] = len(mats)
            mats.append(L(2, sp) - L(2 - di, sm))
        else:
            idx[(di, dj, "A")] = len(mats)
            mats.append(L(2, sp))
            idx[(di, dj, "B")] = len(mats)
            mats.append(L(2 - di, sm))
            idx[(di, dj, "C")] = len(mats)
            mats.append(L(2 - di, -sm))
    arr = np.stack(mats, 1)  # [NG, n_mats, NOUT]
    return arr.astype(ml_dtypes.bfloat16), idx


N_MATS = 1 + 2 * 2 + 10 * 3  # 35


def _reflect_runs(v0, v1, h):
    """Split virtual row range [v0, v1] into runs of physical rows.
    Returns list of (p_offset, phys_start, count, step) with step +-1."""
    runs = []
    v = v0
    while v <= v1:
        if v < 0:
            e = min(-1, v1)
            runs.append((v - v0, -v, e - v + 1, -1))
            v = e + 1
        elif v >= h:
            e = v1
            runs.append((v - v0, 2 * h - 2 - v, e - v + 1, -1))
            v = e + 1
        else:
            e = min(h - 1, v1)
            runs.append((v - v0, v, e - v + 1, 1))
            v = e + 1
    return runs


def build_bilateral(nc, s0, mat_idx, h=H, w=W, b_sh=B_SH, c=C):
    """Emit the per-core program. s0 = spatial[2,2] (center weight)."""
    img_d = nc.dram_tensor("images", [b_sh, c, h, w], f32, kind="ExternalInput").ap()
    shifts_d = nc.dram_tensor(
        "shifts", [NG, N_MATS, NOUT], bf16, kind="ExternalInput"
    ).ap()
    out_d = nc.dram_tensor("out", [b_sh, c, h, w], bf16, kind="ExternalOutput").ap()

    # const APs for activation biases (0.0 for derf, s0 for Identity-add)
    for val in sorted({0.0, float(s0)}):
        key = (f32, val)
        if key not in nc.const_aps.aps:
            t = nc.alloc_sbuf_tensor(f"cbias-{val}", [128, 1], f32)
            nc.gpsimd.memset(t.ap(), val)
            nc.const_aps.aps[key] = t.ap()
    nc.all_engine_barrier()

    with tile.TileContext(nc) as tc, ExitStack() as ctx:
        consts = ctx.enter_context(tc.tile_pool(name="consts", bufs=1))
        imgs_f = ctx.enter_context(tc.tile_pool(name="imgs_f", bufs=2))
        imgs_b = ctx.enter_context(tc.tile_pool(name="imgs_b", bufs=2))
        dpool = ctx.enter_context(tc.tile_pool(name="dpool", bufs=3))
        gpool = ctx.enter_context(tc.tile_pool(name="gpool", bufs=3))
        upool = ctx.enter_context(tc.tile_pool(name="upool", bufs=3))
        finals = ctx.enter_context(tc.tile_pool(name="finals", bufs=1))
        psums = ctx.enter_context(tc.tile_pool(name="psums", bufs=1, space="PSUM"))

        shifts = consts.tile([NG, N_MATS, NOUT], bf16)

        # Units: 4 full 124-row bands per image (no overlap), then one
        # shared tail unit computing both images' last 16 rows at once
        # (img0 at plane partitions 0.., img1 at 32..). The banded lhsT
        # matrices are reused by column-slicing [:, idx, 0:mtot].
        units = []
        for bi in range(b_sh):
            for r0 in range(0, h - NOUT - 15, NOUT):
                units.append([(bi, r0, 0, NOUT)])
        tail_r0 = (h // NOUT) * NOUT  # 496
        units.append([(ii, tail_r0, 32 * ii, h - tail_r0) for ii in range(b_sh)])

        pending_drain = None
        pending_drain_vec = None
        for ui, segs in enumerate(units):
            mtot = max(pb + nr for (_, _, pb, nr) in segs)
            if True:
                # ---- load 3 row-shifted reflect-padded f32 image copies ----
                # First unit: nothing else runs yet, so alternate the load
                # DMAs across both HWDGE rings (SP + ACT) to double the
                # in-flight transfer budget and halve the cold-start ramp.
                nld = 0
                ifs = []
                for s in range(3):
                    t = imgs_f.tile([NG, c, WB], f32, tag=f"i{s}f")
                    for (bi, r0, pb, nr) in segs:
                        for (po, ps, cnt, step) in _reflect_runs(
                            r0 - 2 + s, r0 + s + nr + 1, h
                        ):
                            if step == 1:
                                # per-channel DMAs: each instruction rides
                                # one SDMA engine (~27 GiB/s), so splitting
                                # puts 3 engines on the tile; first unit
                                # splits rows too (cold start, empty lanes)
                                # row-split only at pipeline-restart units
                                # (cold DMA lanes); elsewhere fewer, larger
                                # DMAs keep the issue queue shorter
                                rsp = 2 if (r0 == 0 and cnt > 32) else 1
                                for ch in range(c):
                                    for ri in range(rsp):
                                        ra = po + (cnt * ri) // rsp
                                        rb = po + (cnt * (ri + 1)) // rsp
                                        sa = ps + (cnt * ri) // rsp
                                        ldq = (
                                            nc.scalar
                                            if ui == 0 and nld % 2 == 1
                                            else nc.sync
                                        )
                                        nld += 1
                                        ldq.dma_start(
                                            t[pb + ra : pb + rb, ch, 2 : 2 + w],
                                            img_d[bi, ch, sa : sa + rb - ra, :],
                                        )
                            else:
                                # reflected rows: load straight from DRAM
                                for k in range(cnt):
                                    nc.sync.dma_start(
                                        t[pb + po + k : pb + po + k + 1, :, 2 : 2 + w],
                                        img_d[
                                            bi, :, ps - k : ps - k + 1, :
                                        ].rearrange("c r n -> r c n"),
                                    )
                    # reflect pad cols (image cols -2,-1,512,513), tiny DVE
                    for (j, jsrc) in ((0, 4), (1, 3), (2 + w, w), (3 + w, w - 1)):
                        nc.vector.tensor_copy(
                            t[:, :, j : j + 1], t[:, :, jsrc : jsrc + 1]
                        )
                    ifs.append(t)
                    if ui == 0 and s == 1:
                        # the 1.1MB weights load, chunked (one dma_start
                        # rides one SDMA engine), emitted only now so the
                        # first unit's image loads win the issue queue
                        for i in range(0, N_MATS, 5):
                            j2 = min(i + 5, N_MATS)
                            nc.sync.dma_start(
                                shifts[:, i:j2, :], shifts_d[:, i:j2, :]
                            )

                ibA, ibB = [], []
                for s in range(3):
                    a = imgs_b.tile([NG, c, WB], bf16, tag=f"i{s}bA")
                    nc.vector.tensor_copy(a[:], ifs[s][:])
                    ibA.append(a)
                for s in range(3):
                    b = imgs_b.tile([NG, c, WB], bf16, tag=f"i{s}bB")
                    for ch in range(c):
                        nc.sync.dma_start(
                            b[:, ch, 0 : WB - 1], ibA[s][:, ch, 1:WB]
                        )
                    ibB.append(b)

                # all subs issued up-front so the in-order DVE queue never
                # blocks a later batch's subs behind an earlier batch's umult
                dtiles = []
                for bt, batch in enumerate(BATCHES):
                    d = dpool.tile([NG, NB * c, WIN], bf16, tag="d")
                    for sl, (di, dj) in enumerate(batch):
                        jb = _jbase(dj)
                        cen = ibA[0][:, :, 2 + jb : 2 + jb + WIN]
                        if dj % 2 == 0:
                            sh = ibA[di][:, :, 2 + jb + dj : 2 + jb + dj + WIN]
                        else:
                            sh = ibB[di][:, :, 1 + jb + dj : 1 + jb + dj + WIN]
                        dsl = d[:, sl * c : (sl + 1) * c, :]
                        nc.vector.tensor_tensor(dsl, sh, cen, Alu.subtract)
                    dtiles.append(d)

                # ---- PSUM accumulators ----
                pw = psums.tile([NOUT, c, 512], f32, tag="pw")
                pa = psums.tile([NOUT, c, 512], f32, tag="pa")

                # matmuls per psum bank (channel)
                n_pw_ch = 2 * 1 + 10 * 2
                n_pa_ch = 2 * 1 + 10 * 2
                pw_cnt = [0] * c
                pa_cnt = [0] * c
                deferred_pa = []

                for bt, batch in enumerate(BATCHES):
                    d = dtiles[bt]
                    g = gpool.tile([NG, NB * c, WIN], bf16, tag="g")
                    u = upool.tile([NG, NB * c, WIN], bf16, tag="u")
                    # half-batch ACT + umult for finer pipelining; the very
                    # first batch of the kernel runs per-pair ACTs so the
                    # first matmul is gated on a 1.6us ACT, not a 2.9us one
                    hh = NB * c // 2
                    if ui == 0 and bt == 0:
                        for q in range(0, hh, c):
                            nc.scalar.activation(
                                g[:, q : q + c, :], d[:, q : q + c, :],
                                AF.Derivative_Erf, bias=0.0, scale=SQ50,
                            )
                    else:
                        nc.scalar.activation(
                            g[:, 0:hh, :], d[:, 0:hh, :],
                            AF.Derivative_Erf, bias=0.0, scale=SQ50,
                        )
                    if bt == 0 and pending_drain is not None:
                        # previous unit's scalar-side drain (recip frees the
                        # pw banks, pasb frees pa) goes right behind this
                        # unit's first derf half: early PSUM release without
                        # its matmul-stop wait blocking the derfs
                        pending_drain()
                        pending_drain = None
                    nc.scalar.activation(
                        g[:, hh:, :], d[:, hh:, :],
                        AF.Derivative_Erf, bias=0.0, scale=SQ50,
                    )
                    nc.vector.tensor_tensor(
                        u[:, 0:hh, :], g[:, 0:hh, :], d[:, 0:hh, :], Alu.mult
                    )
                    nc.vector.tensor_tensor(
                        u[:, hh:, :], g[:, hh:, :], d[:, hh:, :], Alu.mult
                    )
                    if bt == 0 and pending_drain_vec is not None:
                        # previous unit's vector-side finals + stores go
                        # after this unit's u-mults on the DVE queue
                        pending_drain_vec()
                        pending_drain_vec = None

                    # ---- PE accumulation streams ----
                    # start/stop are per PSUM zero-region (= per channel bank)
                    def mm_pw(mat, rhs, ch):
                        k = pw_cnt[ch]
                        nc.tensor.matmul(
                            pw[0:mtot, ch, :], mat, rhs,
                            start=k == 0, stop=k == n_pw_ch - 1,
                        )
                        pw_cnt[ch] = k + 1

                    def mm_pa(mat, rhs, ch):
                        k = pa_cnt[ch]
                        nc.tensor.matmul(
                            pa[0:mtot, ch, :], mat, rhs,
                            start=k == 0, stop=k == n_pa_ch - 1,
                        )
                        pa_cnt[ch] = k + 1

                    # pw (g-dependent) streams first: PE can start right
                    # after the ACT, overlapping the u-mults on DVE
                    for sl, (di, dj) in enumerate(batch):
                        jb = _jbase(dj)
                        od = -jb            # direct window offset in plane
                        os_ = -jb - dj      # shifted window offset
                        if dj == 0:
                            for rhs_ch in range(c):
                                mm_pw(shifts[:, mat_idx[(di, dj, "den")], 0:mtot],
                                      g[:, sl * c + rhs_ch, od : od + 512],
                                      rhs_ch)
                        else:
                            for rhs_ch in range(c):
                                mm_pw(shifts[:, mat_idx[(di, dj, "A")], 0:mtot],
                                      g[:, sl * c + rhs_ch, od : od + 512],
                                      rhs_ch)
                            for rhs_ch in range(c):
                                mm_pw(shifts[:, mat_idx[(di, dj, "B")], 0:mtot],
                                      g[:, sl * c + rhs_ch, os_ : os_ + 512],
                                      rhs_ch)

                    def emit_pa(batch, u):
                        for sl, (di, dj) in enumerate(batch):
                            jb = _jbase(dj)
                            od = -jb
                            os_ = -jb - dj
                            if dj == 0:
                                for rhs_ch in range(c):
                                    mm_pa(
                                        shifts[:, mat_idx[(di, dj, "num")], 0:mtot],
                                        u[:, sl * c + rhs_ch, od : od + 512],
                                        rhs_ch)
                            else:
                                for rhs_ch in range(c):
                                    mm_pa(
                                        shifts[:, mat_idx[(di, dj, "A")], 0:mtot],
                                        u[:, sl * c + rhs_ch, od : od + 512],
                                        rhs_ch)
                                for rhs_ch in range(c):
                                    mm_pa(
                                        shifts[:, mat_idx[(di, dj, "C")], 0:mtot],
                                        u[:, sl * c + rhs_ch, os_ : os_ + 512],
                                        rhs_ch)

                    if ui == len(units) - 1:
                        # last unit: defer all pa streams behind all pw
                        # streams so the reciprocal (gated on pw) overlaps
                        # the remaining matmuls -> shorter end-of-kernel tail
                        deferred_pa.append((batch, u))
                        if bt == len(BATCHES) - 1:
                            for b_, u_ in deferred_pa:
                                emit_pa(b_, u_)
                    else:
                        emit_pa(batch, u)

                # ---- drain + finals, deferred into the next unit's
                # emission (or flushed at the end). Scalar phase first
                # (frees PSUM banks), vector phase later.
                def make_drain(pw, pa, cen, segs, mtot):
                    state = {}

                    def drain_scalar():
                        # rec = 1/(pw+s0) and pasb = bf16(pa) on ScalarE
                        rec = finals.tile([NOUT, c, 512], bf16, tag="rec")
                        _act_recip(nc, rec[0:mtot], pw[0:mtot], float(s0))
                        pasb = finals.tile([NOUT, c, 512], bf16, tag="pasb")
                        nc.scalar.copy(pasb[0:mtot], pa[0:mtot])
                        state["rec"], state["pasb"] = rec, pasb

                    def drain_vec():
                        rec, pasb = state["rec"], state["pasb"]
                        # out = p + pa * rec
                        res = finals.tile([NOUT, c, 512], bf16, tag="res")
                        nc.vector.tensor_tensor(
                            res[0:mtot], pasb[0:mtot], rec[0:mtot], Alu.mult
                        )
                        outp = finals.tile([NOUT, c, 512], bf16, tag="outp", bufs=2)
                        nc.vector.tensor_tensor(
                            outp[0:mtot], cen[0:mtot], res[0:mtot], Alu.add
                        )
                        for (bi, r0, pb, nr) in segs:
                            for ch in range(c):
                                nc.sync.dma_start(
                                    out_d[bi, ch, r0 : r0 + nr, :],
                                    outp[pb : pb + nr, ch, :],
                                )
                    return drain_scalar, drain_vec

                pending_drain, pending_drain_vec = make_drain(
                    pw, pa, ibA[2][:, :, 2 : 2 + w], segs, mtot
                )
        if pending_drain is not None:
            pending_drain()
        if pending_drain_vec is not None:
            pending_drain_vec()
    return nc


def make_program(spatial_kernel):
    spatial_kernel = np.asarray(spatial_kernel, dtype=np.float32)
    mats, mat_idx = _shift_mats(spatial_kernel)
    s0 = float(spatial_kernel[2, 2])
    _restrict_act_tables()
    nc = bacc.Bacc("TRN2", target_bir_lowering=False, debug=False)
    build_bilateral(nc, s0, mat_idx)
    nc.compile()
    return nc, mats


def kernel(images, spatial_kernel):
    images = np.asarray(images, dtype=np.float32)
    spatial_kernel = np.asarray(spatial_kernel, dtype=np.float32)
    nc, mats = make_program(spatial_kernel)
    in_maps = [
        {"images": images[i * B_SH : (i + 1) * B_SH], "shifts": mats}
        for i in range(N_CORES)
    ]
    res = run_bass_kernel_spmd(nc, in_maps, core_ids=list(range(N_CORES)))
    return np.concatenate(
        [res.results[i]["out"].astype(np.float32) for i in range(N_CORES)], axis=0
    )


# revision 54
# speedup vs baseline: 1.0201x; 1.0132x over previous
"""Bilateral anti-alias filter on Trainium2, 8-core data parallel.

Full inputs: images [16,3,512,512] f32, spatial_kernel [5,5] f32.
Shards the batch over 8 NeuronCores (2 images each), runs a Bass/Tile
kernel per core, gathers the full output.

Math (per pixel, K=5, sigma_i=0.1), using pair symmetry over the 12
offsets t=(di,dj) with di>=0 lexicographically positive:

  d_t = p(x+t) - p(x)
  e_t = exp(-50 d_t^2)  computed as Derivative_Erf(sqrt(50) d)*sqrt(pi)/2
  u_t = e_t * d_t
  den(x) = s0 + sum_t [s+_t e_t + s-_t shift_t(e_t)]
  pa(x)  = sum_t [s+_t u_t - s-_t shift_t(u_t)]
  out = p + pa / den

shift_t realized on the TensorEngine via banded lhsT matmuls into PSUM
(spatial weights folded into the lhsT values).  Each core runs 9 band
units: 4 full 124-row bands per image plus one shared tail unit that
computes both images' last 16 rows at once.  PSUM drains through ScalarE
(Reciprocal table + bf16 copy), finals are bf16 on DVE, and the output
is stored bf16 and upcast on the host.
"""
import sys

sys.path.insert(0, "/opt/trn_rl_repo")

import math
import numpy as np
import ml_dtypes
from contextlib import ExitStack

import concourse.bass as bass
import concourse.tile as tile
from concourse import bacc, mybir
from concourse.bass_utils import run_bass_kernel_spmd

f32 = mybir.dt.float32
bf16 = mybir.dt.bfloat16
AF = mybir.ActivationFunctionType
Alu = mybir.AluOpType

N_CORES = 8
B_FULL, C, H, W = 16, 3, 512, 512
B_SH = B_FULL // N_CORES  # 2 images per core
KK = 5
PAD = KK // 2  # 2
SQ50 = float(np.sqrt(np.float32(50.0)))
C_DERF = 2.0 / math.sqrt(math.pi)  # Derivative_Erf(x) = C_DERF*exp(-x^2)
NOUT = 124  # output rows per band
NG = 128    # plane partitions (= NOUT + 4)
WB = W + 4  # 516: padded col buffer, tile col c <-> image col c-2
WIN = W + 2  # 514: per-pair plane window width

# 12 pairs (di, dj) with di >= 0, lexicographically positive.
# Ordered so each batch's FIRST half holds even-dj pairs (which read only
# the ibA copies): the first ACT half-batch never waits on the ibB DMAs.
PAIRS = [
    (0, 2), (1, -2), (0, 1), (1, -1),
    (1, 0), (1, 2), (1, 1), (2, -1),
    (2, -2), (2, 0), (2, 1), (2, 2),
]
BATCHES = [PAIRS[0:4], PAIRS[4:8], PAIRS[8:12]]
NB = 4  # pairs per batch


def _jbase(dj):
    """Image col of plane-window col 0 (window covers jbase..jbase+513)."""
    return -2 if dj > 0 else 0


def _act_recip(nc, out, in_, bias):
    """rec = 1/(in_ + bias) via the ACT Reciprocal table (bass's public
    activation() refuses Reciprocal; its accuracy is ~1e-5 rel on our
    [1, 10] domain, fine for this kernel's 2e-2 budget)."""
    bias_ap = nc.const_aps.scalar_like(float(bias), in_)
    ins = [
        nc.scalar.lower_ap(in_),
        nc.scalar.lower_ap(bias_ap),
        mybir.ImmediateValue(dtype=f32, value=1.0),  # scale
        mybir.ImmediateValue(dtype=f32, value=0.0),  # alpha
    ]
    return nc.scalar.add_instruction(
        mybir.InstActivation(
            name=nc.get_next_instruction_name(),
            func=AF.Reciprocal,
            ins=ins,
            outs=[nc.scalar.lower_ap(out)],
        )
    )


def _restrict_act_tables():
    """Steer the activation-table chooser so per-band table swaps stay at
    2 (derf set <-> recip set): keep every set (indices into act_info.json
    must be preserved) but strip Derivative_Erf/Reciprocal membership from
    all other sets so they can't be chosen for them."""
    import concourse.bacc as cbacc

    if getattr(cbacc.get_activation_tables, "_bilateral_patched", False):
        return
    orig = cbacc.get_activation_tables
    keep = {
        "erf_derivative",
        "natural_log_exp_and_others",
        "reciprocal_and_small",
    }
    strip = {AF.Exp, AF.Ln, AF.Derivative_Erf, AF.Reciprocal}

    def patched(arch):
        tabs = orig(arch)
        return {
            k: (set(v) if k in keep else set(v) - strip)
            for k, v in tabs.items()
        }

    patched._bilateral_patched = True
    cbacc.get_activation_tables = patched


def _shift_mats(spatial):
    """Banded lhsT matrices [NG, n_mats, NOUT] bf16 with spatial weights
    (divided by C_DERF) folded in. Returns (array, {(di,dj,kind): idx})."""
    def L(k, scale):
        a = np.zeros((NG, NOUT), np.float64)
        for m in range(NOUT):
            a[m + k, m] = scale
        return a

    mats, idx = [], {}
    idx["s0"] = 0
    mats.append(L(2, float(spatial[2, 2])))  # ones-stream: den += s0
    for (di, dj) in PAIRS:
        sp = float(spatial[2 + di, 2 + dj]) / C_DERF
        sm = float(spatial[2 - di, 2 - dj]) / C_DERF
        if dj == 0:
            idx[(di, dj, "den")] = len(mats)
            mats.append(L(2, sp) + L(2 - di, sm))
            idx[(di, dj, "num")] = len(mats)
            mats.append(L(2, sp) - L(2 - di, sm))
        else:
            idx[(di, dj, "A")] = len(mats)
            mats.append(L(2, sp))
            idx[(di, dj, "B")] = len(mats)
            mats.append(L(2 - di, sm))
            idx[(di, dj, "C")] = len(mats)
            mats.append(L(2 - di, -sm))
    arr = np.stack(mats, 1)  # [NG, n_mats, NOUT]
    return arr.astype(ml_dtypes.bfloat16), idx


N_MATS = 1 + 2 * 2 + 10 * 3  # 35


def _reflect_runs(v0, v1, h):
    """Split virtual row range [v0, v1] into runs of physical rows.
    Returns list of (p_offset, phys_start, count, step) with step +-1."""
    runs = []
    v = v0
    while v <= v1:
        if v < 0:
            e = min(-1, v1)
            runs.append((v - v0, -v, e - v + 1, -1))
            v = e + 1
        elif v >= h:
            e = v1
            runs.append((v - v0, 2 * h - 2 - v, e - v + 1, -1))
            v = e + 1
        else:
            e = min(h - 1, v1)
            runs.append((v - v0, v, e - v + 1, 1))
            v = e + 1
    return runs


def build_bilateral(nc, s0, mat_idx, h=H, w=W, b_sh=B_SH, c=C):
    """Emit the per-core program. s0 = spatial[2,2] (center weight)."""
    img_d = nc.dram_tensor("images", [b_sh, c, h, w], f32, kind="ExternalInput").ap()
    shifts_d = nc.dram_tensor(
        "shifts", [NG, N_MATS, NOUT], bf16, kind="ExternalInput"
    ).ap()
    out_d = nc.dram_tensor("out", [b_sh, c, h, w], bf16, kind="ExternalOutput").ap()

    # const APs for activation biases (0.0 for derf, s0 for Identity-add)
    for val in sorted({0.0, float(s0)}):
        key = (f32, val)
        if key not in nc.const_aps.aps:
            t = nc.alloc_sbuf_tensor(f"cbias-{val}", [128, 1], f32)
            nc.gpsimd.memset(t.ap(), val)
            nc.const_aps.aps[key] = t.ap()
    nc.all_engine_barrier()

    with tile.TileContext(nc) as tc, ExitStack() as ctx:
        consts = ctx.enter_context(tc.tile_pool(name="consts", bufs=1))
        imgs_f = ctx.enter_context(tc.tile_pool(name="imgs_f", bufs=2))
        imgs_b = ctx.enter_context(tc.tile_pool(name="imgs_b", bufs=2))
        dpool = ctx.enter_context(tc.tile_pool(name="dpool", bufs=3))
        gpool = ctx.enter_context(tc.tile_pool(name="gpool", bufs=3))
        upool = ctx.enter_context(tc.tile_pool(name="upool", bufs=3))
        finals = ctx.enter_context(tc.tile_pool(name="finals", bufs=1))
        psums = ctx.enter_context(tc.tile_pool(name="psums", bufs=1, space="PSUM"))

        shifts = consts.tile([NG, N_MATS, NOUT], bf16)

        # Units: 4 full 124-row bands per image (no overlap), then one
        # shared tail unit computing both images' last 16 rows at once
        # (img0 at plane partitions 0.., img1 at 32..). The banded lhsT
        # matrices are reused by column-slicing [:, idx, 0:mtot].
        units = []
        for bi in range(b_sh):
            for r0 in range(0, h - NOUT - 15, NOUT):
                units.append([(bi, r0, 0, NOUT)])
        tail_r0 = (h // NOUT) * NOUT  # 496
        units.append([(ii, tail_r0, 32 * ii, h - tail_r0) for ii in range(b_sh)])

        pending_drain = None
        pending_drain_vec = None
        for ui, segs in enumerate(units):
            mtot = max(pb + nr for (_, _, pb, nr) in segs)
            if True:
                # ---- load 3 row-shifted reflect-padded f32 image copies ----
                # First unit: nothing else runs yet, so alternate the load
                # DMAs across both HWDGE rings (SP + ACT) to double the
                # in-flight transfer budget and halve the cold-start ramp.
                nld = 0
                ifs, ibA, ibB = [], [], []

                def load_plane(s):
                    nonlocal nld
                    t = imgs_f.tile([NG, c, WB], f32, tag=f"i{s}f", name=f"t{s}")
                    for (bi, r0, pb, nr) in segs:
                        for (po, ps, cnt, step) in _reflect_runs(
                            r0 - 2 + s, r0 + s + nr + 1, h
                        ):
                            if step == 1:
                                # per-channel DMAs: each instruction rides
                                # one SDMA engine (~27 GiB/s); row-split too
                                # at pipeline-restart units (cold DMA lanes)
                                rsp = 2 if (r0 == 0 and cnt > 32) else 1
                                for ch in range(c):
                                    for ri in range(rsp):
                                        ra = po + (cnt * ri) // rsp
                                        rb = po + (cnt * (ri + 1)) // rsp
                                        sa = ps + (cnt * ri) // rsp
                                        ldq = (
                                            nc.scalar
                                            if ui == 0 and nld % 2 == 1
                                            else nc.sync
                                        )
                                        nld += 1
                                        ldq.dma_start(
                                            t[pb + ra : pb + rb, ch, 2 : 2 + w],
                                            img_d[bi, ch, sa : sa + rb - ra, :],
                                        )
                            else:
                                # reflected rows: load straight from DRAM
                                for k in range(cnt):
                                    nc.sync.dma_start(
                                        t[pb + po + k : pb + po + k + 1, :, 2 : 2 + w],
                                        img_d[
                                            bi, :, ps - k : ps - k + 1, :
                                        ].rearrange("c r n -> r c n"),
                                    )
                    # reflect pad cols (image cols -2,-1,512,513), tiny DVE
                    for (j, jsrc) in ((0, 4), (1, 3), (2 + w, w), (3 + w, w - 1)):
                        nc.vector.tensor_copy(
                            t[:, :, j : j + 1], t[:, :, jsrc : jsrc + 1]
                        )
                    ifs.append(t)

                def cast_and_shift(s):
                    a = imgs_b.tile([NG, c, WB], bf16, tag=f"i{s}bA", name=f"a{s}")
                    nc.vector.tensor_copy(a[:], ifs[s][:])
                    ibA.append(a)
                    b = imgs_b.tile([NG, c, WB], bf16, tag=f"i{s}bB", name=f"b{s}")
                    for ch in range(c):
                        nc.sync.dma_start(
                            b[:, ch, 0 : WB - 1], a[:, ch, 1:WB]
                        )
                    ibB.append(b)

                def load_shift_mats():
                    # the 1.1MB weights load, chunked (one dma_start rides
                    # one SDMA engine), emitted late so the first unit's
                    # image loads win the issue queue
                    for i in range(0, N_MATS, 5):
                        j2 = min(i + 5, N_MATS)
                        nc.sync.dma_start(
                            shifts[:, i:j2, :], shifts_d[:, i:j2, :]
                        )

                if ui == 0:
                    # cold start: s2 loads go AFTER the s0/s1 column-shift
                    # copies on the issue queue, so the first batch's odd-dj
                    # subs (which need ibB[0..1]) aren't stuck behind them
                    load_plane(0)
                    load_plane(1)
                    load_shift_mats()
                    cast_and_shift(0)
                    cast_and_shift(1)
                    load_plane(2)
                    cast_and_shift(2)
                else:
                    for s in range(3):
                        load_plane(s)
                    for s in range(3):
                        cast_and_shift(s)

                # all subs issued up-front so the in-order DVE queue never
                # blocks a later batch's subs behind an earlier batch's umult
                dtiles = []
                for bt, batch in enumerate(BATCHES):
                    d = dpool.tile([NG, NB * c, WIN], bf16, tag="d")
                    for sl, (di, dj) in enumerate(batch):
                        jb = _jbase(dj)
                        cen = ibA[0][:, :, 2 + jb : 2 + jb + WIN]
                        if dj % 2 == 0:
                            sh = ibA[di][:, :, 2 + jb + dj : 2 + jb + dj + WIN]
                        else:
                            sh = ibB[di][:, :, 1 + jb + dj : 1 + jb + dj + WIN]
                        dsl = d[:, sl * c : (sl + 1) * c, :]
                        nc.vector.tensor_tensor(dsl, sh, cen, Alu.subtract)
                    dtiles.append(d)

                # ---- PSUM accumulators ----
                pw = psums.tile([NOUT, c, 512], f32, tag="pw")
                pa = psums.tile([NOUT, c, 512], f32, tag="pa")

                # matmuls per psum bank (channel)
                n_pw_ch = 2 * 1 + 10 * 2
                n_pa_ch = 2 * 1 + 10 * 2
                pw_cnt = [0] * c
                pa_cnt = [0] * c
                deferred_pa = []

                for bt, batch in enumerate(BATCHES):
                    d = dtiles[bt]
                    g = gpool.tile([NG, NB * c, WIN], bf16, tag="g")
                    u = upool.tile([NG, NB * c, WIN], bf16, tag="u")
                    # half-batch ACT + umult for finer pipelining; the very
                    # first batch of the kernel runs per-pair ACTs so the
                    # first matmul is gated on a 1.6us ACT, not a 2.9us one
                    hh = NB * c // 2
                    if ui == 0 and bt == 0:
                        for q in range(0, hh, c):
                            nc.scalar.activation(
                                g[:, q : q + c, :], d[:, q : q + c, :],
                                AF.Derivative_Erf, bias=0.0, scale=SQ50,
                            )
                    else:
                        nc.scalar.activation(
                            g[:, 0:hh, :], d[:, 0:hh, :],
                            AF.Derivative_Erf, bias=0.0, scale=SQ50,
                        )
                    if bt == 0 and pending_drain is not None:
                        # previous unit's scalar-side drain (recip frees the
                        # pw banks, pasb frees pa) goes right behind this
                        # unit's first derf half: early PSUM release without
                        # its matmul-stop wait blocking the derfs
                        pending_drain()
                        pending_drain = None
                    nc.scalar.activation(
                        g[:, hh:, :], d[:, hh:, :],
                        AF.Derivative_Erf, bias=0.0, scale=SQ50,
                    )
                    nc.vector.tensor_tensor(
                        u[:, 0:hh, :], g[:, 0:hh, :], d[:, 0:hh, :], Alu.mult
                    )
                    nc.vector.tensor_tensor(
                        u[:, hh:, :], g[:, hh:, :], d[:, hh:, :], Alu.mult
                    )
                    if bt == 0 and pending_drain_vec is not None:
                        # previous unit's vector-side finals + stores go
                        # after this unit's u-mults on the DVE queue
                        pending_drain_vec()
                        pending_drain_vec = None

                    # ---- PE accumulation streams ----
                    # start/stop are per PSUM zero-region (= per channel bank)
                    def mm_pw(mat, rhs, ch):
                        k = pw_cnt[ch]
                        nc.tensor.matmul(
                            pw[0:mtot, ch, :], mat, rhs,
                            start=k == 0, stop=k == n_pw_ch - 1,
                        )
                        pw_cnt[ch] = k + 1

                    def mm_pa(mat, rhs, ch):
                        k = pa_cnt[ch]
                        nc.tensor.matmul(
                            pa[0:mtot, ch, :], mat, rhs,
                            start=k == 0, stop=k == n_pa_ch - 1,
                        )
                        pa_cnt[ch] = k + 1

                    # pw (g-dependent) streams first: PE can start right
                    # after the ACT, overlapping the u-mults on DVE
                    for sl, (di, dj) in enumerate(batch):
                        jb = _jbase(dj)
                        od = -jb            # direct window offset in plane
                        os_ = -jb - dj      # shifted window offset
                        if dj == 0:
                            for rhs_ch in range(c):
                                mm_pw(shifts[:, mat_idx[(di, dj, "den")], 0:mtot],
                                      g[:, sl * c + rhs_ch, od : od + 512],
                                      rhs_ch)
                        else:
                            for rhs_ch in range(c):
                                mm_pw(shifts[:, mat_idx[(di, dj, "A")], 0:mtot],
                                      g[:, sl * c + rhs_ch, od : od + 512],
                                      rhs_ch)
                            for rhs_ch in range(c):
                                mm_pw(shifts[:, mat_idx[(di, dj, "B")], 0:mtot],
                                      g[:, sl * c + rhs_ch, os_ : os_ + 512],
                                      rhs_ch)

                    def emit_pa(batch, u, chans):
                        for sl, (di, dj) in enumerate(batch):
                            jb = _jbase(dj)
                            od = -jb
                            os_ = -jb - dj
                            if dj == 0:
                                for rhs_ch in chans:
                                    mm_pa(
                                        shifts[:, mat_idx[(di, dj, "num")], 0:mtot],
                                        u[:, sl * c + rhs_ch, od : od + 512],
                                        rhs_ch)
                            else:
                                for rhs_ch in chans:
                                    mm_pa(
                                        shifts[:, mat_idx[(di, dj, "A")], 0:mtot],
                                        u[:, sl * c + rhs_ch, od : od + 512],
                                        rhs_ch)
                                for rhs_ch in chans:
                                    mm_pa(
                                        shifts[:, mat_idx[(di, dj, "C")], 0:mtot],
                                        u[:, sl * c + rhs_ch, os_ : os_ + 512],
                                        rhs_ch)

                    if ui == len(units) - 1:
                        # last unit: defer all pa streams behind all pw
                        # streams, then emit them CHANNEL-MAJOR so ch0's
                        # drain/finals overlap ch1/ch2's matmuls -> the
                        # end-of-kernel tail is only ch2's chain
                        deferred_pa.append((batch, u))
                        if bt == len(BATCHES) - 1:
                            for ch in range(c):
                                for b_, u_ in deferred_pa:
                                    emit_pa(b_, u_, [ch])
                    else:
                        emit_pa(batch, u, range(c))

                # ---- drain + finals, deferred into the next unit's
                # emission (or flushed at the end). Scalar phase first
                # (frees PSUM banks), vector phase later. per_ch=True
                # (last unit) interleaves each channel's drain with the
                # remaining channels' matmuls.
                def make_drain(pw, pa, cen, segs, mtot, per_ch):
                    state = {}

                    def drain_scalar():
                        # rec = 1/(pw+s0) and pasb = bf16(pa) on ScalarE
                        rec = finals.tile([NOUT, c, 512], bf16, tag="rec")
                        _act_recip(nc, rec[0:mtot], pw[0:mtot], float(s0))
                        pasb = finals.tile([NOUT, c, 512], bf16, tag="pasb")
                        if per_ch:
                            for ch in range(c):
                                nc.scalar.copy(
                                    pasb[0:mtot, ch, :], pa[0:mtot, ch, :]
                                )
                        else:
                            nc.scalar.copy(pasb[0:mtot], pa[0:mtot])
                        state["rec"], state["pasb"] = rec, pasb

                    def drain_vec():
                        rec, pasb = state["rec"], state["pasb"]
                        # out = p + pa * rec
                        res = finals.tile([NOUT, c, 512], bf16, tag="res")
                        outp = finals.tile([NOUT, c, 512], bf16, tag="outp", bufs=2)
                        if per_ch:
                            for ch in range(c):
                                nc.vector.tensor_tensor(
                                    res[0:mtot, ch, :], pasb[0:mtot, ch, :],
                                    rec[0:mtot, ch, :], Alu.mult,
                                )
                                nc.vector.tensor_tensor(
                                    outp[0:mtot, ch, :], cen[0:mtot, ch, :],
                                    res[0:mtot, ch, :], Alu.add,
                                )
                                for (bi, r0, pb, nr) in segs:
                                    nc.sync.dma_start(
                                        out_d[bi, ch, r0 : r0 + nr, :],
                                        outp[pb : pb + nr, ch, :],
                                    )
                        else:
                            nc.vector.tensor_tensor(
                                res[0:mtot], pasb[0:mtot], rec[0:mtot], Alu.mult
                            )
                            nc.vector.tensor_tensor(
                                outp[0:mtot], cen[0:mtot], res[0:mtot], Alu.add
                            )
                            for (bi, r0, pb, nr) in segs:
                                for ch in range(c):
                                    nc.sync.dma_start(
                                        out_d[bi, ch, r0 : r0 + nr, :],
                                        outp[pb : pb + nr, ch, :],
                                    )
                    return drain_scalar, drain_vec

                pending_drain, pending_drain_vec = make_drain(
                    pw, pa, ibA[2][:, :, 2 : 2 + w], segs, mtot,
                    ui == len(units) - 1,
                )
        if pending_drain is not None:
            pending_drain()
        if pending_drain_vec is not None:
            pending_drain_vec()
    return nc


def make_program(spatial_kernel):
    spatial_kernel = np.asarray(spatial_kernel, dtype=np.float32)
    mats, mat_idx = _shift_mats(spatial_kernel)
    s0 = float(spatial_kernel[2, 2])
    _restrict_act_tables()
    nc = bacc.Bacc("TRN2", target_bir_lowering=False, debug=False)
    build_bilateral(nc, s0, mat_idx)
    nc.compile()
    return nc, mats


def kernel(images, spatial_kernel):
    images = np.asarray(images, dtype=np.float32)
    spatial_kernel = np.asarray(spatial_kernel, dtype=np.float32)
    nc, mats = make_program(spatial_kernel)
    in_maps = [
        {"images": images[i * B_SH : (i + 1) * B_SH], "shifts": mats}
        for i in range(N_CORES)
    ]
    res = run_bass_kernel_spmd(nc, in_maps, core_ids=list(range(N_CORES)))
    return np.concatenate(
        [res.results[i]["out"].astype(np.float32) for i in range(N_CORES)], axis=0
    )


# revision 56
# speedup vs baseline: 1.0212x; 1.0011x over previous
"""Bilateral anti-alias filter on Trainium2, 8-core data parallel.

Full inputs: images [16,3,512,512] f32, spatial_kernel [5,5] f32.
Shards the batch over 8 NeuronCores (2 images each), runs a Bass/Tile
kernel per core, gathers the full output.

Math (per pixel, K=5, sigma_i=0.1), using pair symmetry over the 12
offsets t=(di,dj) with di>=0 lexicographically positive:

  d_t = p(x+t) - p(x)
  e_t = exp(-50 d_t^2)  computed as Derivative_Erf(sqrt(50) d)*sqrt(pi)/2
  u_t = e_t * d_t
  den(x) = s0 + sum_t [s+_t e_t + s-_t shift_t(e_t)]
  pa(x)  = sum_t [s+_t u_t - s-_t shift_t(u_t)]
  out = p + pa / den

shift_t realized on the TensorEngine via banded lhsT matmuls into PSUM
(spatial weights folded into the lhsT values).  Each core runs 9 band
units: 4 full 124-row bands per image plus one shared tail unit that
computes both images' last 16 rows at once.  PSUM drains through ScalarE
(Reciprocal table + bf16 copy), finals are bf16 on DVE, and the output
is stored bf16 and upcast on the host.
"""
import sys

sys.path.insert(0, "/opt/trn_rl_repo")

import math
import numpy as np
import ml_dtypes
from contextlib import ExitStack

import concourse.bass as bass
import concourse.tile as tile
from concourse import bacc, mybir
from concourse.bass_utils import run_bass_kernel_spmd

f32 = mybir.dt.float32
bf16 = mybir.dt.bfloat16
AF = mybir.ActivationFunctionType
Alu = mybir.AluOpType

N_CORES = 8
B_FULL, C, H, W = 16, 3, 512, 512
B_SH = B_FULL // N_CORES  # 2 images per core
KK = 5
PAD = KK // 2  # 2
SQ50 = float(np.sqrt(np.float32(50.0)))
C_DERF = 2.0 / math.sqrt(math.pi)  # Derivative_Erf(x) = C_DERF*exp(-x^2)
NOUT = 124  # output rows per band
NG = 128    # plane partitions (= NOUT + 4)
WB = W + 4  # 516: padded col buffer, tile col c <-> image col c-2
WIN = W + 2  # 514: per-pair plane window width

# 12 pairs (di, dj) with di >= 0, lexicographically positive.
# Ordered so each batch's FIRST half holds even-dj pairs (which read only
# the ibA copies): the first ACT half-batch never waits on the ibB DMAs.
PAIRS = [
    (0, 2), (1, -2), (0, 1), (1, -1),
    (1, 0), (1, 2), (1, 1), (2, -1),
    (2, -2), (2, 0), (2, 1), (2, 2),
]
BATCHES = [PAIRS[0:4], PAIRS[4:8], PAIRS[8:12]]
NB = 4  # pairs per batch


def _jbase(dj):
    """Image col of plane-window col 0 (window covers jbase..jbase+513)."""
    return -2 if dj > 0 else 0


def _act_recip(nc, out, in_, bias):
    """rec = 1/(in_ + bias) via the ACT Reciprocal table (bass's public
    activation() refuses Reciprocal; its accuracy is ~1e-5 rel on our
    [1, 10] domain, fine for this kernel's 2e-2 budget)."""
    bias_ap = nc.const_aps.scalar_like(float(bias), in_)
    ins = [
        nc.scalar.lower_ap(in_),
        nc.scalar.lower_ap(bias_ap),
        mybir.ImmediateValue(dtype=f32, value=1.0),  # scale
        mybir.ImmediateValue(dtype=f32, value=0.0),  # alpha
    ]
    return nc.scalar.add_instruction(
        mybir.InstActivation(
            name=nc.get_next_instruction_name(),
            func=AF.Reciprocal,
            ins=ins,
            outs=[nc.scalar.lower_ap(out)],
        )
    )


def _restrict_act_tables():
    """Steer the activation-table chooser so per-band table swaps stay at
    2 (derf set <-> recip set): keep every set (indices into act_info.json
    must be preserved) but strip Derivative_Erf/Reciprocal membership from
    all other sets so they can't be chosen for them."""
    import concourse.bacc as cbacc

    if getattr(cbacc.get_activation_tables, "_bilateral_patched", False):
        return
    orig = cbacc.get_activation_tables
    keep = {
        "erf_derivative",
        "natural_log_exp_and_others",
        "reciprocal_and_small",
    }
    strip = {AF.Exp, AF.Ln, AF.Derivative_Erf, AF.Reciprocal}

    def patched(arch):
        tabs = orig(arch)
        return {
            k: (set(v) if k in keep else set(v) - strip)
            for k, v in tabs.items()
        }

    patched._bilateral_patched = True
    cbacc.get_activation_tables = patched


def _shift_mats(spatial):
    """Banded lhsT matrices [NG, n_mats, NOUT] bf16 with spatial weights
    (divided by C_DERF) folded in. Returns (array, {(di,dj,kind): idx})."""
    def L(k, scale):
        a = np.zeros((NG, NOUT), np.float64)
        for m in range(NOUT):
            a[m + k, m] = scale
        return a

    mats, idx = [], {}
    idx["s0"] = 0
    mats.append(L(2, float(spatial[2, 2])))  # ones-stream: den += s0
    for (di, dj) in PAIRS:
        sp = float(spatial[2 + di, 2 + dj]) / C_DERF
        sm = float(spatial[2 - di, 2 - dj]) / C_DERF
        if dj == 0:
            idx[(di, dj, "den")] = len(mats)
            mats.append(L(2, sp) + L(2 - di, sm))
            idx[(di, dj, "num")] = len(mats)
            mats.append(L(2, sp) - L(2 - di, sm))
        else:
            idx[(di, dj, "A")] = len(mats)
            mats.append(L(2, sp))
            idx[(di, dj, "B")] = len(mats)
            mats.append(L(2 - di, sm))
            idx[(di, dj, "C")] = len(mats)
            mats.append(L(2 - di, -sm))
    arr = np.stack(mats, 1)  # [NG, n_mats, NOUT]
    return arr.astype(ml_dtypes.bfloat16), idx


N_MATS = 1 + 2 * 2 + 10 * 3  # 35


def _reflect_runs(v0, v1, h):
    """Split virtual row range [v0, v1] into runs of physical rows.
    Returns list of (p_offset, phys_start, count, step) with step +-1."""
    runs = []
    v = v0
    while v <= v1:
        if v < 0:
            e = min(-1, v1)
            runs.append((v - v0, -v, e - v + 1, -1))
            v = e + 1
        elif v >= h:
            e = v1
            runs.append((v - v0, 2 * h - 2 - v, e - v + 1, -1))
            v = e + 1
        else:
            e = min(h - 1, v1)
            runs.append((v - v0, v, e - v + 1, 1))
            v = e + 1
    return runs


def build_bilateral(nc, s0, mat_idx, h=H, w=W, b_sh=B_SH, c=C):
    """Emit the per-core program. s0 = spatial[2,2] (center weight)."""
    img_d = nc.dram_tensor("images", [b_sh, c, h, w], f32, kind="ExternalInput").ap()
    shifts_d = nc.dram_tensor(
        "shifts", [NG, N_MATS, NOUT], bf16, kind="ExternalInput"
    ).ap()
    out_d = nc.dram_tensor("out", [b_sh, c, h, w], bf16, kind="ExternalOutput").ap()

    # const APs for activation biases (0.0 for derf, s0 for Identity-add)
    for val in sorted({0.0, float(s0)}):
        key = (f32, val)
        if key not in nc.const_aps.aps:
            t = nc.alloc_sbuf_tensor(f"cbias-{val}", [128, 1], f32)
            nc.gpsimd.memset(t.ap(), val)
            nc.const_aps.aps[key] = t.ap()
    nc.all_engine_barrier()

    with tile.TileContext(nc) as tc, ExitStack() as ctx:
        consts = ctx.enter_context(tc.tile_pool(name="consts", bufs=1))
        imgs_f = ctx.enter_context(tc.tile_pool(name="imgs_f", bufs=2))
        imgs_b = ctx.enter_context(tc.tile_pool(name="imgs_b", bufs=2))
        dpool = ctx.enter_context(tc.tile_pool(name="dpool", bufs=3))
        gpool = ctx.enter_context(tc.tile_pool(name="gpool", bufs=3))
        upool = ctx.enter_context(tc.tile_pool(name="upool", bufs=3))
        finals = ctx.enter_context(tc.tile_pool(name="finals", bufs=1))
        psums = ctx.enter_context(tc.tile_pool(name="psums", bufs=1, space="PSUM"))

        shifts = consts.tile([NG, N_MATS, NOUT], bf16)

        # PE warm-up: the HAM clock gate starts at 1.2 GHz and needs ~3.4us
        # of sustained matmul activity to release to 2.4 GHz. Run dummy
        # matmuls on (uninitialized, dependency-free) SBUF into a scratch
        # PSUM bank during the load ramp so the first real matmuls start
        # warm instead of paying ~4us of cold issue.
        warm = consts.tile([NG, 768], bf16)
        nc.gpsimd.memset(warm[:], 0.5)
        wpsum = psums.tile([NOUT, 512], f32, tag="warm")
        for _ in range(60):
            nc.tensor.matmul(
                wpsum[:], warm[:, 0:NOUT], warm[:, 256:768],
                start=True, stop=True,
            )

        # Units: 4 full 124-row bands per image (no overlap), then one
        # shared tail unit computing both images' last 16 rows at once
        # (img0 at plane partitions 0.., img1 at 32..). The banded lhsT
        # matrices are reused by column-slicing [:, idx, 0:mtot].
        units = []
        for bi in range(b_sh):
            for r0 in range(0, h - NOUT - 15, NOUT):
                units.append([(bi, r0, 0, NOUT)])
        tail_r0 = (h // NOUT) * NOUT  # 496
        units.append([(ii, tail_r0, 32 * ii, h - tail_r0) for ii in range(b_sh)])

        pending_drain = None
        pending_drain_vec = None
        for ui, segs in enumerate(units):
            mtot = max(pb + nr for (_, _, pb, nr) in segs)
            if True:
                # ---- load 3 row-shifted reflect-padded f32 image copies ----
                # First unit: nothing else runs yet, so alternate the load
                # DMAs across both HWDGE rings (SP + ACT) to double the
                # in-flight transfer budget and halve the cold-start ramp.
                nld = 0
                ifs, ibA, ibB = [], [], []

                def load_plane(s):
                    nonlocal nld
                    t = imgs_f.tile([NG, c, WB], f32, tag=f"i{s}f", name=f"t{s}")
                    for (bi, r0, pb, nr) in segs:
                        for (po, ps, cnt, step) in _reflect_runs(
                            r0 - 2 + s, r0 + s + nr + 1, h
                        ):
                            if step == 1:
                                # per-channel DMAs: each instruction rides
                                # one SDMA engine (~27 GiB/s); row-split too
                                # at pipeline-restart units (cold DMA lanes)
                                rsp = 2 if (r0 == 0 and cnt > 32) else 1
                                for ch in range(c):
                                    for ri in range(rsp):
                                        ra = po + (cnt * ri) // rsp
                                        rb = po + (cnt * (ri + 1)) // rsp
                                        sa = ps + (cnt * ri) // rsp
                                        ldq = (
                                            nc.scalar
                                            if ui == 0 and nld % 2 == 1
                                            else nc.sync
                                        )
                                        nld += 1
                                        ldq.dma_start(
                                            t[pb + ra : pb + rb, ch, 2 : 2 + w],
                                            img_d[bi, ch, sa : sa + rb - ra, :],
                                        )
                            else:
                                # reflected rows: load straight from DRAM
                                for k in range(cnt):
                                    nc.sync.dma_start(
                                        t[pb + po + k : pb + po + k + 1, :, 2 : 2 + w],
                                        img_d[
                                            bi, :, ps - k : ps - k + 1, :
                                        ].rearrange("c r n -> r c n"),
                                    )
                    # reflect pad cols (image cols -2,-1,512,513), tiny DVE
                    for (j, jsrc) in ((0, 4), (1, 3), (2 + w, w), (3 + w, w - 1)):
                        nc.vector.tensor_copy(
                            t[:, :, j : j + 1], t[:, :, jsrc : jsrc + 1]
                        )
                    ifs.append(t)

                def cast_and_shift(s):
                    a = imgs_b.tile([NG, c, WB], bf16, tag=f"i{s}bA", name=f"a{s}")
                    nc.vector.tensor_copy(a[:], ifs[s][:])
                    ibA.append(a)
                    b = imgs_b.tile([NG, c, WB], bf16, tag=f"i{s}bB", name=f"b{s}")
                    for ch in range(c):
                        nc.sync.dma_start(
                            b[:, ch, 0 : WB - 1], a[:, ch, 1:WB]
                        )
                    ibB.append(b)

                def load_shift_mats():
                    # the 1.1MB weights load, chunked (one dma_start rides
                    # one SDMA engine), emitted late so the first unit's
                    # image loads win the issue queue
                    for i in range(0, N_MATS, 5):
                        j2 = min(i + 5, N_MATS)
                        nc.sync.dma_start(
                            shifts[:, i:j2, :], shifts_d[:, i:j2, :]
                        )

                if ui == 0:
                    # cold start: s2 loads go AFTER the s0/s1 column-shift
                    # copies on the issue queue, so the first batch's odd-dj
                    # subs (which need ibB[0..1]) aren't stuck behind them
                    load_plane(0)
                    load_plane(1)
                    load_shift_mats()
                    cast_and_shift(0)
                    cast_and_shift(1)
                    load_plane(2)
                    cast_and_shift(2)
                else:
                    for s in range(3):
                        load_plane(s)
                    for s in range(3):
                        cast_and_shift(s)

                # all subs issued up-front so the in-order DVE queue never
                # blocks a later batch's subs behind an earlier batch's umult
                dtiles = []
                for bt, batch in enumerate(BATCHES):
                    d = dpool.tile([NG, NB * c, WIN], bf16, tag="d")
                    for sl, (di, dj) in enumerate(batch):
                        jb = _jbase(dj)
                        cen = ibA[0][:, :, 2 + jb : 2 + jb + WIN]
                        if dj % 2 == 0:
                            sh = ibA[di][:, :, 2 + jb + dj : 2 + jb + dj + WIN]
                        else:
                            sh = ibB[di][:, :, 1 + jb + dj : 1 + jb + dj + WIN]
                        dsl = d[:, sl * c : (sl + 1) * c, :]
                        nc.vector.tensor_tensor(dsl, sh, cen, Alu.subtract)
                    dtiles.append(d)

                # ---- PSUM accumulators ----
                pw = psums.tile([NOUT, c, 512], f32, tag="pw")
                pa = psums.tile([NOUT, c, 512], f32, tag="pa")

                # matmuls per psum bank (channel)
                n_pw_ch = 2 * 1 + 10 * 2
                n_pa_ch = 2 * 1 + 10 * 2
                pw_cnt = [0] * c
                pa_cnt = [0] * c
                deferred_pa = []

                for bt, batch in enumerate(BATCHES):
                    d = dtiles[bt]
                    g = gpool.tile([NG, NB * c, WIN], bf16, tag="g")
                    u = upool.tile([NG, NB * c, WIN], bf16, tag="u")
                    # half-batch ACT + umult for finer pipelining; the very
                    # first batch of the kernel runs per-pair ACTs so the
                    # first matmul is gated on a 1.6us ACT, not a 2.9us one
                    hh = NB * c // 2
                    if ui == 0 and bt == 0:
                        for q in range(0, hh, c):
                            nc.scalar.activation(
                                g[:, q : q + c, :], d[:, q : q + c, :],
                                AF.Derivative_Erf, bias=0.0, scale=SQ50,
                            )
                    else:
                        nc.scalar.activation(
                            g[:, 0:hh, :], d[:, 0:hh, :],
                            AF.Derivative_Erf, bias=0.0, scale=SQ50,
                        )
                    if bt == 0 and pending_drain is not None:
                        # previous unit's scalar-side drain (recip frees the
                        # pw banks, pasb frees pa) goes right behind this
                        # unit's first derf half: early PSUM release without
                        # its matmul-stop wait blocking the derfs
                        pending_drain()
                        pending_drain = None
                    nc.scalar.activation(
                        g[:, hh:, :], d[:, hh:, :],
                        AF.Derivative_Erf, bias=0.0, scale=SQ50,
                    )
                    nc.vector.tensor_tensor(
                        u[:, 0:hh, :], g[:, 0:hh, :], d[:, 0:hh, :], Alu.mult
                    )
                    nc.vector.tensor_tensor(
                        u[:, hh:, :], g[:, hh:, :], d[:, hh:, :], Alu.mult
                    )
                    if bt == 0 and pending_drain_vec is not None:
                        # previous unit's vector-side finals + stores go
                        # after this unit's u-mults on the DVE queue
                        pending_drain_vec()
                        pending_drain_vec = None

                    # ---- PE accumulation streams ----
                    # start/stop are per PSUM zero-region (= per channel bank)
                    def mm_pw(mat, rhs, ch):
                        k = pw_cnt[ch]
                        nc.tensor.matmul(
                            pw[0:mtot, ch, :], mat, rhs,
                            start=k == 0, stop=k == n_pw_ch - 1,
                        )
                        pw_cnt[ch] = k + 1

                    def mm_pa(mat, rhs, ch):
                        k = pa_cnt[ch]
                        nc.tensor.matmul(
                            pa[0:mtot, ch, :], mat, rhs,
                            start=k == 0, stop=k == n_pa_ch - 1,
                        )
                        pa_cnt[ch] = k + 1

                    # pw (g-dependent) streams first: PE can start right
                    # after the ACT, overlapping the u-mults on DVE
                    for sl, (di, dj) in enumerate(batch):
                        jb = _jbase(dj)
                        od = -jb            # direct window offset in plane
                        os_ = -jb - dj      # shifted window offset
                        if dj == 0:
                            for rhs_ch in range(c):
                                mm_pw(shifts[:, mat_idx[(di, dj, "den")], 0:mtot],
                                      g[:, sl * c + rhs_ch, od : od + 512],
                                      rhs_ch)
                        else:
                            for rhs_ch in range(c):
                                mm_pw(shifts[:, mat_idx[(di, dj, "A")], 0:mtot],
                                      g[:, sl * c + rhs_ch, od : od + 512],
                                      rhs_ch)
                            for rhs_ch in range(c):
                                mm_pw(shifts[:, mat_idx[(di, dj, "B")], 0:mtot],
                                      g[:, sl * c + rhs_ch, os_ : os_ + 512],
                                      rhs_ch)

                    def emit_pa(batch, u, chans):
                        for sl, (di, dj) in enumerate(batch):
                            jb = _jbase(dj)
                            od = -jb
                            os_ = -jb - dj
                            if dj == 0:
                                for rhs_ch in chans:
                                    mm_pa(
                                        shifts[:, mat_idx[(di, dj, "num")], 0:mtot],
                                        u[:, sl * c + rhs_ch, od : od + 512],
                                        rhs_ch)
                            else:
                                for rhs_ch in chans:
                                    mm_pa(
                                        shifts[:, mat_idx[(di, dj, "A")], 0:mtot],
                                        u[:, sl * c + rhs_ch, od : od + 512],
                                        rhs_ch)
                                for rhs_ch in chans:
                                    mm_pa(
                                        shifts[:, mat_idx[(di, dj, "C")], 0:mtot],
                                        u[:, sl * c + rhs_ch, os_ : os_ + 512],
                                        rhs_ch)

                    if ui == len(units) - 1:
                        # last unit: defer all pa streams behind all pw
                        # streams, then emit them CHANNEL-MAJOR so ch0's
                        # drain/finals overlap ch1/ch2's matmuls -> the
                        # end-of-kernel tail is only ch2's chain
                        deferred_pa.append((batch, u))
                        if bt == len(BATCHES) - 1:
                            for ch in range(c):
                                for b_, u_ in deferred_pa:
                                    emit_pa(b_, u_, [ch])
                    else:
                        emit_pa(batch, u, range(c))

                # ---- drain + finals, deferred into the next unit's
                # emission (or flushed at the end). Scalar phase first
                # (frees PSUM banks), vector phase later. per_ch=True
                # (last unit) interleaves each channel's drain with the
                # remaining channels' matmuls.
                def make_drain(pw, pa, cen, segs, mtot, per_ch):
                    state = {}

                    def drain_scalar():
                        # rec = 1/(pw+s0) and pasb = bf16(pa) on ScalarE
                        rec = finals.tile([NOUT, c, 512], bf16, tag="rec")
                        _act_recip(nc, rec[0:mtot], pw[0:mtot], float(s0))
                        pasb = finals.tile([NOUT, c, 512], bf16, tag="pasb")
                        if per_ch:
                            for ch in range(c):
                                nc.scalar.copy(
                                    pasb[0:mtot, ch, :], pa[0:mtot, ch, :]
                                )
                        else:
                            nc.scalar.copy(pasb[0:mtot], pa[0:mtot])
                        state["rec"], state["pasb"] = rec, pasb

                    def drain_vec():
                        rec, pasb = state["rec"], state["pasb"]
                        # out = p + pa * rec
                        res = finals.tile([NOUT, c, 512], bf16, tag="res")
                        outp = finals.tile([NOUT, c, 512], bf16, tag="outp", bufs=2)
                        if per_ch:
                            for ch in range(c):
                                nc.vector.tensor_tensor(
                                    res[0:mtot, ch, :], pasb[0:mtot, ch, :],
                                    rec[0:mtot, ch, :], Alu.mult,
                                )
                                nc.vector.tensor_tensor(
                                    outp[0:mtot, ch, :], cen[0:mtot, ch, :],
                                    res[0:mtot, ch, :], Alu.add,
                                )
                                for (bi, r0, pb, nr) in segs:
                                    nc.sync.dma_start(
                                        out_d[bi, ch, r0 : r0 + nr, :],
                                        outp[pb : pb + nr, ch, :],
                                    )
                        else:
                            nc.vector.tensor_tensor(
                                res[0:mtot], pasb[0:mtot], rec[0:mtot], Alu.mult
                            )
                            nc.vector.tensor_tensor(
                                outp[0:mtot], cen[0:mtot], res[0:mtot], Alu.add
                            )
                            for (bi, r0, pb, nr) in segs:
                                for ch in range(c):
                                    nc.sync.dma_start(
                                        out_d[bi, ch, r0 : r0 + nr, :],
                                        outp[pb : pb + nr, ch, :],
                                    )
                    return drain_scalar, drain_vec

                pending_drain, pending_drain_vec = make_drain(
                    pw, pa, ibA[2][:, :, 2 : 2 + w], segs, mtot,
                    ui == len(units) - 1,
                )
        if pending_drain is not None:
            pending_drain()
        if pending_drain_vec is not None:
            pending_drain_vec()
    return nc


def make_program(spatial_kernel):
    spatial_kernel = np.asarray(spatial_kernel, dtype=np.float32)
    mats, mat_idx = _shift_mats(spatial_kernel)
    s0 = float(spatial_kernel[2, 2])
    _restrict_act_tables()
    nc = bacc.Bacc("TRN2", target_bir_lowering=False, debug=False)
    build_bilateral(nc, s0, mat_idx)
    nc.compile()
    return nc, mats


def kernel(images, spatial_kernel):
    images = np.asarray(images, dtype=np.float32)
    spatial_kernel = np.asarray(spatial_kernel, dtype=np.float32)
    nc, mats = make_program(spatial_kernel)
    in_maps = [
        {"images": images[i * B_SH : (i + 1) * B_SH], "shifts": mats}
        for i in range(N_CORES)
    ]
    res = run_bass_kernel_spmd(nc, in_maps, core_ids=list(range(N_CORES)))
    return np.concatenate(
        [res.results[i]["out"].astype(np.float32) for i in range(N_CORES)], axis=0
    )
